# revision 3
# baseline (speedup 1.0000x reference)
"""Bahdanau additive attention on 8 TRN2 NeuronCores (Bass/Tile).

reference math:
  hidden  = query @ W1_w.T + W1_b                      [B, H]
  encoder = einsum("blk,hk->blh", keys, W2_w) + W2_b   [B, L, H]
  act     = relu(hidden[:, None, :] + encoder)         [B, L, H]
  scores  = einsum("blh,h->bl", act, Va_w[0]) + Va_b   [B, L]
  weights = softmax(scores, axis=1)                    [B, L]
  context = einsum("blk,bl->bk", keys, weights)        [B, KEY]
  returns (context, weights)

Sharding: data-parallel over batch, 4 batches per core, weights replicated.

Per-core kernel layout: encoder tiles are [128 H-partitions, 512 L-free],
produced by PE matmuls in float32r (full-rate for N>=512).  The Va
reduction over H is another PE matmul (contraction on partitions).  Softmax
skips max-subtraction (scores are O(10), exp is safe in fp32) and Va_b
(softmax shift-invariant).  Context is accumulated unnormalized in PSUM
from natural-layout keys with exp(score) chunks transposed onto partitions
by PE, then scaled by 1/denominator.
"""

import numpy as np

import concourse.bass as bass
import concourse.mybir as mybir
import concourse.tile as tile
from concourse import bacc
from concourse.bass_utils import run_bass_kernel_spmd

N_CORES = 8
B, L, H, KEY = 32, 2048, 1024, 1280
BS = B // N_CORES            # batches per core
P = 128                      # SBUF partitions
HC = H // P                  # 8 chunks of H
KC = KEY // P                # 10 chunks of KEY
LTS = 512                    # L tile size (free dim per matmul)
LT = L // LTS                # 4 L tiles
LJ = LTS // P                # 4 [128]-chunks per L tile
F32 = mybir.dt.float32
F32R = mybir.dt.float32r

_CACHE = {}


def _build_program():
    nc = bacc.Bacc(
        "TRN2", target_bir_lowering=False, debug=False, num_devices=N_CORES
    )
    d_qT = nc.dram_tensor("qT", [H, BS], F32, kind="ExternalInput").ap()
    d_w1T = nc.dram_tensor("w1T", [H, H], F32, kind="ExternalInput").ap()
    d_biasT = nc.dram_tensor("biasT", [P, HC], F32, kind="ExternalInput").ap()
    d_vaT = nc.dram_tensor("vaT", [P, HC], F32R, kind="ExternalInput").ap()
    d_w2T = nc.dram_tensor("w2T", [KEY, H], F32R, kind="ExternalInput").ap()
    d_keysT = nc.dram_tensor("keysT", [BS, KEY, L], F32R, kind="ExternalInput").ap()
    d_keysN = nc.dram_tensor("keysN", [BS, L, KEY], F32R, kind="ExternalInput").ap()
    d_ctx = nc.dram_tensor("ctx", [BS, KEY], F32, kind="ExternalOutput").ap()
    d_wts = nc.dram_tensor("wts", [BS, L], F32, kind="ExternalOutput").ap()

    AF = mybir.ActivationFunctionType

    with tile.TileContext(nc) as tc:
        with tc.tile_pool(name="consts", bufs=1) as consts:
            biasT_sb = consts.tile([P, HC], F32)
            nc.sync.dma_start(out=biasT_sb, in_=d_biasT)
            vaT_sb = consts.tile([P, HC], F32R)
            nc.sync.dma_start(out=vaT_sb, in_=d_vaT)
            ident1 = consts.tile([1, 1], F32)
            nc.vector.memset(ident1, 1.0)
            # hiddenT[p, hc, b] = (query @ W1_w.T + W1_b + W2_b)[b, hc*128+p]
            hiddT = consts.tile([P, HC, BS], F32)

            w2_pool = tc.tile_pool(name="w2", bufs=1)
            with w2_pool as w2p:
                w2_sb = w2p.tile([P, KC, H], F32R)
                nc.sync.dma_start(
                    out=w2_sb, in_=d_w2T.rearrange("(kc p) h -> p kc h", p=P)
                )

                # ---- stage 1: hiddenT = W1 @ qT + (W1_b + W2_b) ----
                with (
                    tc.tile_pool(name="w1", bufs=1) as w1p,
                    tc.tile_pool(
                        name="ps_h", bufs=2, space=bass.MemorySpace.PSUM
                    ) as ps_h,
                ):
                    w1_sb = w1p.tile([P, HC, H], F32)
                    nc.sync.dma_start(
                        out=w1_sb, in_=d_w1T.rearrange("(ic p) h -> p ic h", p=P)
                    )
                    qT_sb = w1p.tile([P, HC, BS], F32)
                    nc.sync.dma_start(
                        out=qT_sb, in_=d_qT.rearrange("(ic p) b -> p ic b", p=P)
                    )
                    for hc in range(HC):
                        ps = ps_h.tile([P, BS], F32)
                        for ic in range(HC):
                            nc.tensor.matmul(
                                ps,
                                lhsT=w1_sb[:, ic, hc * P : (hc + 1) * P],
                                rhs=qT_sb[:, ic, :],
                                start=(ic == 0),
                                stop=(ic == HC - 1),
                            )
                        nc.vector.tensor_scalar_add(
                            hiddT[:, hc, :], ps, biasT_sb[:, hc : hc + 1]
                        )

                # ---- stage 2: main loop ----
                with (
                    tc.tile_pool(name="kT", bufs=2) as kT_pool,
                    tc.tile_pool(name="kN", bufs=2) as kN_pool,
                    tc.tile_pool(name="act", bufs=3) as act_pool,
                    tc.tile_pool(name="sm", bufs=2) as sm,
                    tc.tile_pool(
                        name="ps_enc", bufs=2, space=bass.MemorySpace.PSUM
                    ) as ps_enc,
                    tc.tile_pool(
                        name="ps_sc", bufs=2, space=bass.MemorySpace.PSUM
                    ) as ps_sc,
                    tc.tile_pool(
                        name="ps_tr", bufs=1, space=bass.MemorySpace.PSUM
                    ) as ps_tr,
                    tc.tile_pool(
                        name="ps_ctx", bufs=1, space=bass.MemorySpace.PSUM
                    ) as ps_ctx,
                ):
                    for b in range(BS):
                        exp_sb = sm.tile([1, L], F32, tag="exp")
                        dparts = sm.tile([1, LT], F32, tag="dparts")
                        ctx_ps = [
                            ps_ctx.tile([1, 512], F32, tag="ctx0", name="ctx0"),
                            ps_ctx.tile([1, 512], F32, tag="ctx1", name="ctx1"),
                            ps_ctx.tile([1, 256], F32, tag="ctx2", name="ctx2"),
                        ]
                        for lt in range(LT):
                            kT = kT_pool.tile([P, KC, LTS], F32R)
                            nc.sync.dma_start(
                                out=kT,
                                in_=d_keysT[b].rearrange("(kc p) l -> p kc l", p=P)[
                                    :, :, lt * LTS : (lt + 1) * LTS
                                ],
                            )
                            kN = kN_pool.tile([P, LJ, KEY], F32R)
                            nc.sync.dma_start(
                                out=kN,
                                in_=d_keysN[b].rearrange("(lc p) k -> p lc k", p=P)[
                                    :, lt * LJ : (lt + 1) * LJ, :
                                ],
                            )
                            sc_ps = ps_sc.tile([1, LTS], F32)
                            for hc in range(HC):
                                e_ps = ps_enc.tile([P, LTS], F32)
                                for kc in range(KC):
                                    nc.tensor.matmul(
                                        e_ps,
                                        lhsT=w2_sb[
                                            :, kc, hc * P : (hc + 1) * P
                                        ],
                                        rhs=kT[:, kc, :],
                                        start=(kc == 0),
                                        stop=(kc == KC - 1),
                                    )
                                a = act_pool.tile([P, LTS], F32R)
                                nc.scalar.activation(
                                    out=a,
                                    in_=e_ps,
                                    func=AF.Relu,
                                    bias=hiddT[:, hc, b : b + 1],
                                    scale=1.0,
                                )
                                nc.tensor.matmul(
                                    sc_ps,
                                    lhsT=vaT_sb[:, hc : hc + 1],
                                    rhs=a,
                                    start=(hc == 0),
                                    stop=(hc == HC - 1),
                                )
                            nc.scalar.activation(
                                out=exp_sb[:, lt * LTS : (lt + 1) * LTS],
                                in_=sc_ps,
                                func=AF.Exp,
                                scale=1.0,
                                accum_out=dparts[:, lt : lt + 1],
                            )
                            tr_ps = ps_tr.tile([P, LJ], F32)
                            for j in range(LJ):
                                lo = lt * LTS + j * P
                                nc.tensor.transpose(
                                    out=tr_ps[:, j : j + 1],
                                    in_=exp_sb[:, lo : lo + P],
                                    identity=ident1,
                                )
                            expT = sm.tile([P, LJ], F32R, tag="expT")
                            nc.vector.tensor_copy(expT, tr_ps)
                            for j in range(LJ):
                                st = lt == 0 and j == 0
                                sp = lt == LT - 1 and j == LJ - 1
                                lw = expT[:, j : j + 1]
                                nc.tensor.matmul(
                                    ctx_ps[0], lhsT=lw,
                                    rhs=kN[:, j, 0:512],
                                    start=st, stop=sp,
                                )
                                nc.tensor.matmul(
                                    ctx_ps[1], lhsT=lw,
                                    rhs=kN[:, j, 512:1024],
                                    start=st, stop=sp,
                                )
                                nc.tensor.matmul(
                                    ctx_ps[2], lhsT=lw,
                                    rhs=kN[:, j, 1024:1280],
                                    start=st, stop=sp,
                                )
                        denom = sm.tile([1, 1], F32, tag="denom")
                        nc.vector.tensor_reduce(
                            denom, dparts,
                            axis=mybir.AxisListType.X, op=mybir.AluOpType.add,
                        )
                        recip = sm.tile([1, 1], F32, tag="recip")
                        nc.vector.reciprocal(recip, denom)
                        wts_sb = sm.tile([1, L], F32, tag="wts")
                        nc.vector.tensor_scalar_mul(wts_sb, exp_sb, recip)
                        nc.sync.dma_start(out=d_wts[b : b + 1, :], in_=wts_sb)
                        ctx_sb = sm.tile([1, KEY], F32, tag="ctxsb")
                        nc.vector.tensor_scalar_mul(
                            ctx_sb[:, 0:512], ctx_ps[0], recip
                        )
                        nc.vector.tensor_scalar_mul(
                            ctx_sb[:, 512:1024], ctx_ps[1], recip
                        )
                        nc.vector.tensor_scalar_mul(
                            ctx_sb[:, 1024:1280], ctx_ps[2], recip
                        )
                        nc.sync.dma_start(out=d_ctx[b : b + 1, :], in_=ctx_sb)

    nc.compile()
    return nc


def _get_nc():
    if "nc" not in _CACHE:
        _CACHE["nc"] = _build_program()
    return _CACHE["nc"]


def _make_in_maps(query, keys, W1_w, W1_b, W2_w, W2_b, Va_w):
    w1T = np.ascontiguousarray(W1_w.T)
    w2T = np.ascontiguousarray(W2_w.T)
    comb = (W1_b + W2_b).astype(np.float32)
    biasT = np.ascontiguousarray(comb.reshape(HC, P).T)
    vaT = np.ascontiguousarray(Va_w[0].reshape(HC, P).T)
    keysT = np.ascontiguousarray(keys.transpose(0, 2, 1))
    keysN = np.ascontiguousarray(keys)
    in_maps = []
    for c in range(N_CORES):
        sl = slice(c * BS, (c + 1) * BS)
        in_maps.append(
            {
                "qT": np.ascontiguousarray(query[sl].T),
                "w1T": w1T,
                "w2T": w2T,
                "biasT": biasT,
                "vaT": vaT,
                "keysT": keysT[sl],
                "keysN": keysN[sl],
            }
        )
    return in_maps


def run(inputs_kw, **spmd_kwargs):
    """Build+run; returns ((context, weights), BassKernelResults)."""
    nc = _get_nc()
    in_maps = _make_in_maps(
        np.asarray(inputs_kw["query"], np.float32),
        np.asarray(inputs_kw["keys"], np.float32),
        np.asarray(inputs_kw["W1_w"], np.float32),
        np.asarray(inputs_kw["W1_b"], np.float32),
        np.asarray(inputs_kw["W2_w"], np.float32),
        np.asarray(inputs_kw["W2_b"], np.float32),
        np.asarray(inputs_kw["Va_w"], np.float32),
    )
    res = run_bass_kernel_spmd(
        nc, in_maps, core_ids=list(range(N_CORES)), **spmd_kwargs
    )
    ctx = np.concatenate([res.results[c]["ctx"] for c in range(N_CORES)], axis=0)
    wts = np.concatenate([res.results[c]["wts"] for c in range(N_CORES)], axis=0)
    return (ctx, wts), res


def kernel(query, keys, W1_w, W1_b, W2_w, W2_b, Va_w, Va_b):
    (ctx, wts), _ = run(
        dict(
            query=query, keys=keys, W1_w=W1_w, W1_b=W1_b,
            W2_w=W2_w, W2_b=W2_b, Va_w=Va_w, Va_b=Va_b,
        )
    )
    return ctx, wts


# revision 5
# speedup vs baseline: 306.4704x; 306.4704x over previous
"""Bahdanau additive attention on 8 TRN2 NeuronCores (Bass/Tile).

reference math:
  hidden  = query @ W1_w.T + W1_b                      [B, H]
  encoder = einsum("blk,hk->blh", keys, W2_w) + W2_b   [B, L, H]
  act     = relu(hidden[:, None, :] + encoder)         [B, L, H]
  scores  = einsum("blh,h->bl", act, Va_w[0]) + Va_b   [B, L]
  weights = softmax(scores, axis=1)                    [B, L]
  context = einsum("blk,bl->bk", keys, weights)        [B, KEY]
  returns (context, weights)

Sharding: data-parallel over batch, 4 batches per core, weights replicated.

Per-core kernel layout: encoder tiles are [128 H-partitions, 512 L-free],
produced by PE matmuls in float32r (full-rate for N>=512).  The Va
reduction over H is another PE matmul (contraction on partitions).  Softmax
skips max-subtraction (scores are O(10), exp is safe in fp32) and Va_b
(softmax shift-invariant).  Context is accumulated unnormalized in PSUM
from natural-layout keys with exp(score) chunks transposed onto partitions
by PE, then scaled by 1/denominator.
"""

import numpy as np

import concourse.bass as bass
import concourse.mybir as mybir
import concourse.tile as tile
from concourse import bacc
from concourse.bass_utils import run_bass_kernel_spmd

N_CORES = 8
B, L, H, KEY = 32, 2048, 1024, 1280
BS = B // N_CORES            # batches per core
P = 128                      # SBUF partitions
HC = H // P                  # 8 chunks of H
KC = KEY // P                # 10 chunks of KEY
LTS = 512                    # L tile size (free dim per matmul)
LT = L // LTS                # 4 L tiles
LJ = LTS // P                # 4 [128]-chunks per L tile
F32 = mybir.dt.float32
F32R = mybir.dt.float32r

_CACHE = {}


def _build_program(repeat=1, variant="full", kt_bufs=2, kn_bufs=2, act_bufs=3):
    nc = bacc.Bacc(
        "TRN2", target_bir_lowering=False, debug=False, num_devices=N_CORES
    )
    d_qT = nc.dram_tensor("qT", [H, BS], F32, kind="ExternalInput").ap()
    d_w1T = nc.dram_tensor("w1T", [H, H], F32, kind="ExternalInput").ap()
    d_biasT = nc.dram_tensor("biasT", [P, HC], F32, kind="ExternalInput").ap()
    d_vaT = nc.dram_tensor("vaT", [P, HC], F32R, kind="ExternalInput").ap()
    d_w2T = nc.dram_tensor("w2T", [KEY, H], F32R, kind="ExternalInput").ap()
    d_keysT = nc.dram_tensor("keysT", [BS, KEY, L], F32R, kind="ExternalInput").ap()
    d_keysN = nc.dram_tensor("keysN", [BS, L, KEY], F32R, kind="ExternalInput").ap()
    d_ctx = nc.dram_tensor("ctx", [BS, KEY], F32, kind="ExternalOutput").ap()
    d_wts = nc.dram_tensor("wts", [BS, L], F32, kind="ExternalOutput").ap()

    AF = mybir.ActivationFunctionType

    with tile.TileContext(nc) as tc:
        with tc.tile_pool(name="consts", bufs=1) as consts:
            biasT_sb = consts.tile([P, HC], F32)
            nc.sync.dma_start(out=biasT_sb, in_=d_biasT)
            vaT_sb = consts.tile([P, HC], F32R)
            nc.sync.dma_start(out=vaT_sb, in_=d_vaT)
            ident1 = consts.tile([1, 1], F32)
            nc.vector.memset(ident1, 1.0)
            # hiddenT[p, hc, b] = (query @ W1_w.T + W1_b + W2_b)[b, hc*128+p]
            hiddT = consts.tile([P, HC, BS], F32)

            w2_pool = tc.tile_pool(name="w2", bufs=1)
            with w2_pool as w2p:
                w2_sb = w2p.tile([P, KC, H], F32R)
                nc.sync.dma_start(
                    out=w2_sb, in_=d_w2T.rearrange("(kc p) h -> p kc h", p=P)
                )

                # ---- stage 1: hiddenT = W1 @ qT + (W1_b + W2_b) ----
                with (
                    tc.tile_pool(name="w1", bufs=1) as w1p,
                    tc.tile_pool(
                        name="ps_h", bufs=2, space=bass.MemorySpace.PSUM
                    ) as ps_h,
                ):
                    w1_sb = w1p.tile([P, HC, H], F32)
                    nc.sync.dma_start(
                        out=w1_sb, in_=d_w1T.rearrange("(ic p) h -> p ic h", p=P)
                    )
                    qT_sb = w1p.tile([P, HC, BS], F32)
                    nc.sync.dma_start(
                        out=qT_sb, in_=d_qT.rearrange("(ic p) b -> p ic b", p=P)
                    )
                    for hc in range(HC):
                        ps = ps_h.tile([P, BS], F32)
                        for ic in range(HC):
                            nc.tensor.matmul(
                                ps,
                                lhsT=w1_sb[:, ic, hc * P : (hc + 1) * P],
                                rhs=qT_sb[:, ic, :],
                                start=(ic == 0),
                                stop=(ic == HC - 1),
                            )
                        nc.vector.tensor_scalar_add(
                            hiddT[:, hc, :], ps, biasT_sb[:, hc : hc + 1]
                        )

                # ---- stage 2: main loop ----
                reps = repeat
                with (
                    tc.tile_pool(name="kT", bufs=kt_bufs) as kT_pool,
                    tc.tile_pool(name="kN", bufs=kn_bufs) as kN_pool,
                    tc.tile_pool(name="act", bufs=act_bufs) as act_pool,
                    tc.tile_pool(name="sm", bufs=2) as sm,
                    tc.tile_pool(
                        name="ps_enc", bufs=2, space=bass.MemorySpace.PSUM
                    ) as ps_enc,
                    tc.tile_pool(
                        name="ps_sc", bufs=2, space=bass.MemorySpace.PSUM
                    ) as ps_sc,
                    tc.tile_pool(
                        name="ps_tr", bufs=1, space=bass.MemorySpace.PSUM
                    ) as ps_tr,
                    tc.tile_pool(
                        name="ps_ctx", bufs=1, space=bass.MemorySpace.PSUM
                    ) as ps_ctx,
                ):
                    for rep in range(reps):
                     for b in range(BS):
                        exp_sb = sm.tile([1, L], F32, tag="exp")
                        dparts = sm.tile([1, LT], F32, tag="dparts")
                        ctx_ps = [
                            ps_ctx.tile([1, 512], F32, tag="ctx0", name="ctx0"),
                            ps_ctx.tile([1, 512], F32, tag="ctx1", name="ctx1"),
                            ps_ctx.tile([1, 256], F32, tag="ctx2", name="ctx2"),
                        ]
                        for lt in range(LT):
                            kT = kT_pool.tile([P, KC, LTS], F32R)
                            nc.sync.dma_start(
                                out=kT,
                                in_=d_keysT[b].rearrange("(kc p) l -> p kc l", p=P)[
                                    :, :, lt * LTS : (lt + 1) * LTS
                                ],
                            )
                            kN = kN_pool.tile([P, LJ, KEY], F32R)
                            nc.sync.dma_start(
                                out=kN,
                                in_=d_keysN[b].rearrange("(lc p) k -> p lc k", p=P)[
                                    :, lt * LJ : (lt + 1) * LJ, :
                                ],
                            )
                            sc_ps = ps_sc.tile([1, LTS], F32)
                            for hc in range(HC):
                                e_ps = ps_enc.tile([P, LTS], F32)
                                for kc in range(KC):
                                    nc.tensor.matmul(
                                        e_ps,
                                        lhsT=w2_sb[
                                            :, kc, hc * P : (hc + 1) * P
                                        ],
                                        rhs=kT[:, kc, :],
                                        start=(kc == 0),
                                        stop=(kc == KC - 1),
                                    )
                                a = act_pool.tile([P, LTS], F32R)
                                nc.scalar.activation(
                                    out=a,
                                    in_=e_ps,
                                    func=AF.Relu,
                                    bias=hiddT[:, hc, b : b + 1],
                                    scale=1.0,
                                )
                                if variant != "enconly":
                                    nc.tensor.matmul(
                                        sc_ps,
                                        lhsT=vaT_sb[:, hc : hc + 1],
                                        rhs=a,
                                        start=(hc == 0),
                                        stop=(hc == HC - 1),
                                    )
                            if variant == "enconly":
                                continue
                            nc.scalar.activation(
                                out=exp_sb[:, lt * LTS : (lt + 1) * LTS],
                                in_=sc_ps,
                                func=AF.Exp,
                                scale=1.0,
                                accum_out=dparts[:, lt : lt + 1],
                            )
                            if variant == "noctx":
                                continue
                            tr_ps = ps_tr.tile([P, LJ], F32)
                            for j in range(LJ):
                                lo = lt * LTS + j * P
                                nc.tensor.transpose(
                                    out=tr_ps[:, j : j + 1],
                                    in_=exp_sb[:, lo : lo + P],
                                    identity=ident1,
                                )
                            expT = sm.tile([P, LJ], F32R, tag="expT")
                            nc.vector.tensor_copy(expT, tr_ps)
                            for j in range(LJ):
                                st = lt == 0 and j == 0
                                sp = lt == LT - 1 and j == LJ - 1
                                lw = expT[:, j : j + 1]
                                nc.tensor.matmul(
                                    ctx_ps[0], lhsT=lw,
                                    rhs=kN[:, j, 0:512],
                                    start=st, stop=sp,
                                )
                                nc.tensor.matmul(
                                    ctx_ps[1], lhsT=lw,
                                    rhs=kN[:, j, 512:1024],
                                    start=st, stop=sp,
                                )
                                nc.tensor.matmul(
                                    ctx_ps[2], lhsT=lw,
                                    rhs=kN[:, j, 1024:1280],
                                    start=st, stop=sp,
                                )
                        if variant == "enconly":
                            continue
                        denom = sm.tile([1, 1], F32, tag="denom")
                        nc.vector.tensor_reduce(
                            denom, dparts,
                            axis=mybir.AxisListType.X, op=mybir.AluOpType.add,
                        )
                        recip = sm.tile([1, 1], F32, tag="recip")
                        nc.vector.reciprocal(recip, denom)
                        wts_sb = sm.tile([1, L], F32, tag="wts")
                        nc.vector.tensor_scalar_mul(wts_sb, exp_sb, recip)
                        nc.sync.dma_start(out=d_wts[b : b + 1, :], in_=wts_sb)
                        if variant == "noctx":
                            continue
                        ctx_sb = sm.tile([1, KEY], F32, tag="ctxsb")
                        nc.vector.tensor_scalar_mul(
                            ctx_sb[:, 0:512], ctx_ps[0], recip
                        )
                        nc.vector.tensor_scalar_mul(
                            ctx_sb[:, 512:1024], ctx_ps[1], recip
                        )
                        nc.vector.tensor_scalar_mul(
                            ctx_sb[:, 1024:1280], ctx_ps[2], recip
                        )
                        nc.sync.dma_start(out=d_ctx[b : b + 1, :], in_=ctx_sb)

    nc.compile()
    return nc


def _get_nc(repeat=1, variant="full", **cfg):
    key = (repeat, variant, tuple(sorted(cfg.items())))
    if key not in _CACHE:
        _CACHE[key] = _build_program(repeat, variant, **cfg)
    return _CACHE[key]


def _make_in_maps(query, keys, W1_w, W1_b, W2_w, W2_b, Va_w):
    w1T = np.ascontiguousarray(W1_w.T)
    w2T = np.ascontiguousarray(W2_w.T)
    comb = (W1_b + W2_b).astype(np.float32)
    biasT = np.ascontiguousarray(comb.reshape(HC, P).T)
    vaT = np.ascontiguousarray(Va_w[0].reshape(HC, P).T)
    keysT = np.ascontiguousarray(keys.transpose(0, 2, 1))
    keysN = np.ascontiguousarray(keys)
    in_maps = []
    for c in range(N_CORES):
        sl = slice(c * BS, (c + 1) * BS)
        in_maps.append(
            {
                "qT": np.ascontiguousarray(query[sl].T),
                "w1T": w1T,
                "w2T": w2T,
                "biasT": biasT,
                "vaT": vaT,
                "keysT": keysT[sl],
                "keysN": keysN[sl],
            }
        )
    return in_maps


def run(inputs_kw, **spmd_kwargs):
    """Build+run; returns ((context, weights), BassKernelResults)."""
    nc = _get_nc()
    in_maps = _make_in_maps(
        np.asarray(inputs_kw["query"], np.float32),
        np.asarray(inputs_kw["keys"], np.float32),
        np.asarray(inputs_kw["W1_w"], np.float32),
        np.asarray(inputs_kw["W1_b"], np.float32),
        np.asarray(inputs_kw["W2_w"], np.float32),
        np.asarray(inputs_kw["W2_b"], np.float32),
        np.asarray(inputs_kw["Va_w"], np.float32),
    )
    res = run_bass_kernel_spmd(
        nc, in_maps, core_ids=list(range(N_CORES)), **spmd_kwargs
    )
    ctx = np.concatenate([res.results[c]["ctx"] for c in range(N_CORES)], axis=0)
    wts = np.concatenate([res.results[c]["wts"] for c in range(N_CORES)], axis=0)
    return (ctx, wts), res


def kernel(query, keys, W1_w, W1_b, W2_w, W2_b, Va_w, Va_b):
    (ctx, wts), _ = run(
        dict(
            query=query, keys=keys, W1_w=W1_w, W1_b=W1_b,
            W2_w=W2_w, W2_b=W2_b, Va_w=Va_w, Va_b=Va_b,
        )
    )
    return ctx, wts


# revision 12
# speedup vs baseline: 324.0355x; 1.0573x over previous
"""Bahdanau additive attention on 8 TRN2 NeuronCores (Bass/Tile).

reference math:
  hidden  = query @ W1_w.T + W1_b                      [B, H]
  encoder = einsum("blk,hk->blh", keys, W2_w) + W2_b   [B, L, H]
  act     = relu(hidden[:, None, :] + encoder)         [B, L, H]
  scores  = einsum("blh,h->bl", act, Va_w[0]) + Va_b   [B, L]
  weights = softmax(scores, axis=1)                    [B, L]
  context = einsum("blk,bl->bk", keys, weights)        [B, KEY]
  returns (context, weights)

Sharding: data-parallel over batch, 4 batches per core, weights replicated.

Per-core kernel layout: encoder tiles are [128 H-partitions, 512 L-free],
produced by PE matmuls in float32r (full-rate for N>=512).  The Va
reduction over H is another PE matmul (contraction on partitions).  Softmax
skips max-subtraction (scores are O(10), exp is safe in fp32) and Va_b
(softmax shift-invariant).  Context is accumulated unnormalized in PSUM
from natural-layout keys with exp(score) chunks transposed onto partitions
by PE, then scaled by 1/denominator.
"""

import numpy as np

import concourse.bass as bass
import concourse.mybir as mybir
import concourse.tile as tile
from concourse import bacc
from concourse.bass_utils import run_bass_kernel_spmd

N_CORES = 8
B, L, H, KEY = 32, 2048, 1024, 1280
BS = B // N_CORES            # batches per core
P = 128                      # SBUF partitions
HC = H // P                  # 8 chunks of H
KC = KEY // P                # 10 chunks of KEY
LTS = 512                    # L tile size (free dim per matmul)
LT = L // LTS                # 4 L tiles
LJ = LTS // P                # 4 [128]-chunks per L tile
F32 = mybir.dt.float32
F32R = mybir.dt.float32r

_CACHE = {}


def _build_program(repeat=1, variant="full", kt_bufs=2, kn_bufs=2, act_bufs=3):
    nc = bacc.Bacc(
        "TRN2", target_bir_lowering=False, debug=False, num_devices=N_CORES
    )
    d_qT = nc.dram_tensor("qT", [H, BS], F32, kind="ExternalInput").ap()
    d_w1T = nc.dram_tensor("w1T", [H, H], F32, kind="ExternalInput").ap()
    d_biasT = nc.dram_tensor("biasT", [P, HC], F32, kind="ExternalInput").ap()
    d_vaT = nc.dram_tensor("vaT", [P, HC], F32R, kind="ExternalInput").ap()
    d_w2T = nc.dram_tensor("w2T", [KEY, H], F32R, kind="ExternalInput").ap()
    d_keysT = nc.dram_tensor("keysT", [BS, KEY, L], F32R, kind="ExternalInput").ap()
    kn_dt = mybir.dt.bfloat16 if variant in ("ctxb16",) else F32R
    d_keysN = nc.dram_tensor("keysN", [BS, L, KEY], kn_dt, kind="ExternalInput").ap()
    d_ctx = nc.dram_tensor("ctx", [BS, KEY], F32, kind="ExternalOutput").ap()
    d_wts = nc.dram_tensor("wts", [BS, L], F32, kind="ExternalOutput").ap()

    AF = mybir.ActivationFunctionType

    with tile.TileContext(nc) as tc:
        with tc.tile_pool(name="consts", bufs=1) as consts:
            biasT_sb = consts.tile([P, HC], F32)
            nc.sync.dma_start(out=biasT_sb, in_=d_biasT)
            vaT_sb = consts.tile([P, HC], F32R)
            nc.sync.dma_start(out=vaT_sb, in_=d_vaT)
            ident1 = consts.tile([1, 1], F32)
            nc.vector.memset(ident1, 1.0)
            # hiddenT[p, hc, b] = (query @ W1_w.T + W1_b + W2_b)[b, hc*128+p]
            hiddT = consts.tile([P, HC, BS], F32)

            w2_pool = tc.tile_pool(name="w2", bufs=1)
            with w2_pool as w2p:
                w2_sb = w2p.tile([P, KC, H], F32R)
                nc.sync.dma_start(
                    out=w2_sb, in_=d_w2T.rearrange("(kc p) h -> p kc h", p=P)
                )

                # ---- stage 1: hiddenT = W1 @ qT + (W1_b + W2_b) ----
                with (
                    tc.tile_pool(name="w1", bufs=1) as w1p,
                    tc.tile_pool(
                        name="ps_h", bufs=2, space=bass.MemorySpace.PSUM
                    ) as ps_h,
                ):
                    w1_sb = w1p.tile([P, HC, H], F32)
                    nc.sync.dma_start(
                        out=w1_sb, in_=d_w1T.rearrange("(ic p) h -> p ic h", p=P)
                    )
                    qT_sb = w1p.tile([P, HC, BS], F32)
                    nc.sync.dma_start(
                        out=qT_sb, in_=d_qT.rearrange("(ic p) b -> p ic b", p=P)
                    )
                    for hc in range(HC):
                        ps = ps_h.tile([P, BS], F32)
                        for ic in range(HC):
                            nc.tensor.matmul(
                                ps,
                                lhsT=w1_sb[:, ic, hc * P : (hc + 1) * P],
                                rhs=qT_sb[:, ic, :],
                                start=(ic == 0),
                                stop=(ic == HC - 1),
                            )
                        nc.vector.tensor_scalar_add(
                            hiddT[:, hc, :], ps, biasT_sb[:, hc : hc + 1]
                        )

                # ---- stage 2: main loop ----
                reps = repeat
                with (
                    tc.tile_pool(name="kT", bufs=kt_bufs) as kT_pool,
                    tc.tile_pool(name="kN", bufs=kn_bufs) as kN_pool,
                    tc.tile_pool(name="act", bufs=act_bufs) as act_pool,
                    tc.tile_pool(name="sm", bufs=2) as sm,
                    tc.tile_pool(
                        name="ps_enc", bufs=2, space=bass.MemorySpace.PSUM
                    ) as ps_enc,
                    tc.tile_pool(
                        name="ps_sc", bufs=2, space=bass.MemorySpace.PSUM
                    ) as ps_sc,
                    tc.tile_pool(
                        name="ps_tr", bufs=1, space=bass.MemorySpace.PSUM
                    ) as ps_tr,
                    tc.tile_pool(
                        name="ps_ctx", bufs=1, space=bass.MemorySpace.PSUM
                    ) as ps_ctx,
                    tc.tile_pool(
                        name="dsc", bufs=2, space=bass.MemorySpace.DRAM
                    ) as dram_sc,
                ):
                    if variant == "dmaonly2":
                        for rep in range(reps):
                            for b in range(BS):
                                for half in range(2):
                                    kT2 = kT_pool.tile(
                                        [P, KC, 1024], F32R, tag="kt2", name="kt2"
                                    )
                                    nc.sync.dma_start(
                                        out=kT2,
                                        in_=d_keysT[b].rearrange(
                                            "(kc p) l -> p kc l", p=P
                                        )[:, :, half * 1024 : (half + 1) * 1024],
                                    )
                                    for q in range(2):
                                        kN2 = kN_pool.tile(
                                            [P, LJ, KEY], kn_dt,
                                            tag="kn", name="kn2",
                                        )
                                        nc.sync.dma_start(
                                            out=kN2,
                                            in_=d_keysN[b].rearrange(
                                                "(lc p) k -> p lc k", p=P
                                            )[
                                                :,
                                                (half * 2 + q) * LJ
                                                : (half * 2 + q + 1) * LJ,
                                                :,
                                            ],
                                        )
                    for rep in range(reps):
                     for b in range(BS):
                        exp_sb = sm.tile([1, L], F32, tag="exp")
                        dparts = sm.tile([1, LT], F32, tag="dparts")
                        if variant == "ctxb16":
                            ctx_acc = ps_ctx.tile(
                                [128, 512], F32, tag="ctxacc", name="ctxacc"
                            )
                            ctx_ps = None
                        else:
                            ctx_ps = [
                                ps_ctx.tile([1, 512], F32, tag="ctx0", name="ctx0"),
                                ps_ctx.tile([1, 512], F32, tag="ctx1", name="ctx1"),
                                ps_ctx.tile([1, 256], F32, tag="ctx2", name="ctx2"),
                            ]
                        if variant == "dmaonly2":
                            continue
                        for lt in range(LT):
                            if variant == "dmaprobe" and lt > 0:
                                pass  # timing probe: reuse lt==0 tiles
                            else:
                                kT = kT_pool.tile([P, KC, LTS], F32R)
                                nc.sync.dma_start(
                                    out=kT,
                                    in_=d_keysT[b].rearrange(
                                        "(kc p) l -> p kc l", p=P
                                    )[:, :, lt * LTS : (lt + 1) * LTS],
                                )
                                kN = kN_pool.tile([P, LJ, KEY], kn_dt)
                                nc.sync.dma_start(
                                    out=kN,
                                    in_=d_keysN[b].rearrange(
                                        "(lc p) k -> p lc k", p=P
                                    )[:, lt * LJ : (lt + 1) * LJ, :],
                                )
                            if variant == "dmaonly":
                                continue
                            sc_ps = ps_sc.tile([1, LTS], F32)
                            for hc in range(HC):
                                e_ps = ps_enc.tile([P, LTS], F32)
                                for kc in range(KC):
                                    nc.tensor.matmul(
                                        e_ps,
                                        lhsT=w2_sb[
                                            :, kc, hc * P : (hc + 1) * P
                                        ],
                                        rhs=kT[:, kc, :],
                                        start=(kc == 0),
                                        stop=(kc == KC - 1),
                                    )
                                a = act_pool.tile([P, LTS], F32R)
                                nc.scalar.activation(
                                    out=a,
                                    in_=e_ps,
                                    func=AF.Relu,
                                    bias=hiddT[:, hc, b : b + 1],
                                    scale=1.0,
                                )
                                if variant != "enconly":
                                    nc.tensor.matmul(
                                        sc_ps,
                                        lhsT=vaT_sb[:, hc : hc + 1],
                                        rhs=a,
                                        start=(hc == 0),
                                        stop=(hc == HC - 1),
                                    )
                            if variant == "enconly":
                                continue
                            nc.scalar.activation(
                                out=exp_sb[:, lt * LTS : (lt + 1) * LTS],
                                in_=sc_ps,
                                func=AF.Exp,
                                scale=1.0,
                                accum_out=dparts[:, lt : lt + 1],
                            )
                            if variant == "noctx":
                                continue
                            tr_ps = ps_tr.tile([P, LJ], F32)
                            for j in range(LJ):
                                lo = lt * LTS + j * P
                                nc.tensor.transpose(
                                    out=tr_ps[:, j : j + 1],
                                    in_=exp_sb[:, lo : lo + P],
                                    identity=ident1,
                                )
                            expT = sm.tile(
                                [P, LJ],
                                mybir.dt.bfloat16 if variant == "ctxb16" else F32R,
                                tag="expT",
                            )
                            nc.vector.tensor_copy(expT, tr_ps)
                            for j in range(LJ):
                                st = lt == 0 and j == 0
                                sp = lt == LT - 1 and j == LJ - 1
                                lw = expT[:, j : j + 1]
                                if variant == "ctxb16":
                                    nc.tensor.matmul(
                                        ctx_acc[0:1, :], lhsT=lw,
                                        rhs=kN[:, j, 0:512],
                                        start=st, stop=sp,
                                        tile_position=(0, 0),
                                    )
                                    nc.tensor.matmul(
                                        ctx_acc[32:33, :], lhsT=lw,
                                        rhs=kN[:, j, 512:1024],
                                        start=st, stop=sp,
                                        tile_position=(0, 32),
                                    )
                                    nc.tensor.matmul(
                                        ctx_acc[64:65, 0:256], lhsT=lw,
                                        rhs=kN[:, j, 1024:1280],
                                        start=st, stop=sp,
                                        tile_position=(0, 64),
                                    )
                                else:
                                    nc.tensor.matmul(
                                        ctx_ps[0], lhsT=lw,
                                        rhs=kN[:, j, 0:512],
                                        start=st, stop=sp,
                                    )
                                    nc.tensor.matmul(
                                        ctx_ps[1], lhsT=lw,
                                        rhs=kN[:, j, 512:1024],
                                        start=st, stop=sp,
                                    )
                                    nc.tensor.matmul(
                                        ctx_ps[2], lhsT=lw,
                                        rhs=kN[:, j, 1024:1280],
                                        start=st, stop=sp,
                                    )
                        if variant in ("enconly", "dmaonly"):
                            continue
                        denom = sm.tile([1, 1], F32, tag="denom")
                        nc.vector.tensor_reduce(
                            denom, dparts,
                            axis=mybir.AxisListType.X, op=mybir.AluOpType.add,
                        )
                        recip = sm.tile([1, 1], F32, tag="recip")
                        nc.vector.reciprocal(recip, denom)
                        wts_sb = sm.tile([1, L], F32, tag="wts")
                        nc.vector.tensor_scalar_mul(wts_sb, exp_sb, recip)
                        nc.sync.dma_start(out=d_wts[b : b + 1, :], in_=wts_sb)
                        if variant == "noctx":
                            continue
                        if variant == "ctxb16":
                            # recip lives on partition 0; broadcast to all
                            # partitions via a DRAM bounce (step-0 partition
                            # source APs are only legal from DRAM)
                            rd = dram_sc.tile([1, 1], F32, tag="rd")
                            nc.gpsimd.dma_start(out=rd, in_=recip)
                            recip3 = sm.tile([P, 1], F32, tag="recip3")
                            rb = bass.AP(
                                tensor=rd.tensor,
                                offset=rd.offset,
                                ap=[[0, P], [1, 1]],
                            )
                            nc.gpsimd.dma_start(out=recip3, in_=rb)
                            ctx_sb3 = sm.tile([P, 512], F32, tag="ctxsb3")
                            for g, n in [(0, 512), (1, 512), (2, 256)]:
                                nc.vector.tensor_scalar_mul(
                                    ctx_sb3[g * 32 : g * 32 + 1, 0:n],
                                    ctx_acc[g * 32 : g * 32 + 1, 0:n],
                                    recip3[g * 32 : g * 32 + 1, :],
                                )
                            nc.sync.dma_start(
                                out=d_ctx[b : b + 1, 0:512], in_=ctx_sb3[0:1, :]
                            )
                            nc.sync.dma_start(
                                out=d_ctx[b : b + 1, 512:1024], in_=ctx_sb3[32:33, :]
                            )
                            nc.sync.dma_start(
                                out=d_ctx[b : b + 1, 1024:1280],
                                in_=ctx_sb3[64:65, 0:256],
                            )
                        else:
                            ctx_sb = sm.tile([1, KEY], F32, tag="ctxsb")
                            nc.vector.tensor_scalar_mul(
                                ctx_sb[:, 0:512], ctx_ps[0], recip
                            )
                            nc.vector.tensor_scalar_mul(
                                ctx_sb[:, 512:1024], ctx_ps[1], recip
                            )
                            nc.vector.tensor_scalar_mul(
                                ctx_sb[:, 1024:1280], ctx_ps[2], recip
                            )
                            nc.sync.dma_start(out=d_ctx[b : b + 1, :], in_=ctx_sb)

    nc.compile()
    return nc


def _get_nc(repeat=1, variant="full", **cfg):
    key = (repeat, variant, tuple(sorted(cfg.items())))
    if key not in _CACHE:
        _CACHE[key] = _build_program(repeat, variant, **cfg)
    return _CACHE[key]


def _make_in_maps(query, keys, W1_w, W1_b, W2_w, W2_b, Va_w, kn_bf16=False):
    w1T = np.ascontiguousarray(W1_w.T)
    w2T = np.ascontiguousarray(W2_w.T)
    comb = (W1_b + W2_b).astype(np.float32)
    biasT = np.ascontiguousarray(comb.reshape(HC, P).T)
    vaT = np.ascontiguousarray(Va_w[0].reshape(HC, P).T)
    keysT = np.ascontiguousarray(keys.transpose(0, 2, 1))
    if kn_bf16:
        import ml_dtypes
        keysN = np.ascontiguousarray(keys.astype(ml_dtypes.bfloat16))
    else:
        keysN = np.ascontiguousarray(keys)
    in_maps = []
    for c in range(N_CORES):
        sl = slice(c * BS, (c + 1) * BS)
        in_maps.append(
            {
                "qT": np.ascontiguousarray(query[sl].T),
                "w1T": w1T,
                "w2T": w2T,
                "biasT": biasT,
                "vaT": vaT,
                "keysT": keysT[sl],
                "keysN": keysN[sl],
            }
        )
    return in_maps


DEFAULT_VARIANT = "ctxb16"


def run(inputs_kw, variant=DEFAULT_VARIANT, **spmd_kwargs):
    """Build+run; returns ((context, weights), BassKernelResults)."""
    nc = _get_nc(1, variant)
    in_maps = _make_in_maps(
        np.asarray(inputs_kw["query"], np.float32),
        np.asarray(inputs_kw["keys"], np.float32),
        np.asarray(inputs_kw["W1_w"], np.float32),
        np.asarray(inputs_kw["W1_b"], np.float32),
        np.asarray(inputs_kw["W2_w"], np.float32),
        np.asarray(inputs_kw["W2_b"], np.float32),
        np.asarray(inputs_kw["Va_w"], np.float32),
        kn_bf16=(variant == "ctxb16"),
    )
    res = run_bass_kernel_spmd(
        nc, in_maps, core_ids=list(range(N_CORES)), **spmd_kwargs
    )
    ctx = np.concatenate([res.results[c]["ctx"] for c in range(N_CORES)], axis=0)
    wts = np.concatenate([res.results[c]["wts"] for c in range(N_CORES)], axis=0)
    return (ctx, wts), res


def kernel(query, keys, W1_w, W1_b, W2_w, W2_b, Va_w, Va_b):
    (ctx, wts), _ = run(
        dict(
            query=query, keys=keys, W1_w=W1_w, W1_b=W1_b,
            W2_w=W2_w, W2_b=W2_b, Va_w=Va_w, Va_b=Va_b,
        )
    )
    return ctx, wts
